# revision 8
# baseline (speedup 1.0000x reference)
"""Depthwise 9x9 same-padding conv (single shared kernel) on Trainium2.

Strategy (per NeuronCore, pure data-parallel over batch across 8 cores):
  - Treat each (b, c) image [256, 256] independently; 256 images per core.
  - fp8e4m3 DoubleRow matmuls (0.5 PE cycles/output-column, 2x the fp32r
    rate) with banded Toeplitz weights contracting over image rows. Each
    DoubleRow matmul carries TWO tap-terms (two (plane, column-shift,
    weight-column) slots) in its 2x128-lane contraction.
  - Precision: X = X8 + Xr8 (fp8 value + fp8 residual), K = K8 + Kr8.
    out = conv(X8, K8) + conv(Xr8, K8) + conv(X8, Kr8) up to ~1e-3. The
    9 conv(X8, K8) terms are essential; the weakest residual terms are
    dropped by a variance budget, the rest packed two-per-matmul.
    For the target inputs this yields 22 terms = 11 matmuls/tile vs the
    9 full-rate fp32r baseline (~1.5e-3 max rel err, gate is 2e-2).
  - HW constraint (measured): the DoubleRow ifmap pair-dim stride must be
    EVEN (odd strides hang the device) and != the 256-column inner read
    (the AP optimizer would merge dims). So slots pair only when their
    column shifts have equal parity; plane pitch is kept even.
  - A 256-row image splits into two 128-row SBUF tiles (out rows 0..123 /
    132..255); cross-tile rows 124..131 of 8 images batch into one
    block-diagonal [128, W] strip. lhsT free dim padded to 128 (walrus
    DoubleRow Ldweights requires 64/128).
  - J images packed per DMA/SBUF tile; fp8 inputs halve input HBM
    traffic. Input DMAs ride the SP HWDGE ring, output DMAs the ACT ring,
    edge/weight DMAs SWDGE.
"""

import numpy as np
import ml_dtypes

import concourse.bass as bass
from concourse import bacc
import concourse.mybir as mybir
import concourse.tile as tile
from concourse.bass_utils import run_bass_kernel_spmd

N_CORES = 8
B, C, H, W = 32, 64, 256, 256
KS, PAD = 9, 4
BC = B // N_CORES          # batches per core
NIMG = BC * C              # images per core
WP = W + 2 * PAD           # padded width 264
MT = 124                   # main out-rows per half-tile (0..123 / 132..255)
EG = 8                     # images per edge-strip group
NGRP = NIMG // EG
J = 4                      # images packed per main DMA/SBUF tile
NBLK = NIMG // J
MM = 128                   # lhsT free columns (walrus DoubleRow needs 64/128)

FP8 = mybir.dt.float8e4
F8NP = ml_dtypes.float8_e4m3
DR = mybir.MatmulPerfMode.DoubleRow

# Residual-slot drop budget: total dropped variance (units of output
# variance) keeping predicted max rel err ~<1.5e-2 with >30% gate margin.
VBUDGET = 0.017

LAST_RESULT = None         # test.py inspects this
LAST_PAIRS = None          # pair plan of the most recent kernel() call


def _bands(kcol, kind, M):
    """[128, M] banded Toeplitz for one 9-tap kernel column `kcol`."""
    Wb = np.zeros((128, M), np.float32)
    if kind == "edge":
        for g in range(EG):
            for m in range(8):
                for u in range(KS):
                    Wb[16 * g + m + u, 8 * g + m] = kcol[u]
        return Wb
    for i in range(MT):
        for u in range(KS):
            ip = i + u - PAD if kind == "top" else i + u
            if 0 <= ip < 128:
                Wb[ip, i] = kcol[u]
    return Wb


def plan_pairs(Kf):
    """Choose kept terms and pair them under the even-stride constraint.

    A term is (plane, v, wsel): plane 0 = X8, 1 = Xr8; v = column shift;
    wsel picks the weight column: ("K8", v), ("Kr8", v) or None (zero).
    Returns a list of ((p1, v1, w1), (p2, v2, w2)) with v1 === v2 (mod 2),
    ordered so the rhs pair stride is positive, even, and never 0 or W.
    """
    K8 = Kf.astype(F8NP).astype(np.float32)
    Kr8 = (Kf - K8).astype(F8NP).astype(np.float32)
    colE8 = (K8**2).sum(axis=0)
    colEr = (Kr8**2).sum(axis=0)
    EXR2 = 7.0e-4            # E[(X - fp8(X))^2] for X ~ N(0,1)

    slots = [("xr", v, EXR2 * colE8[v]) for v in range(KS)]
    slots += [("kr", v, float(colEr[v])) for v in range(KS)]
    slots.sort(key=lambda s: s[2])
    dropped, acc = set(), 0.0
    for t, v, e in slots:
        if acc + e > VBUDGET:
            break
        acc += e
        dropped.add((t, v))

    terms = [(0, v, ("K8", v)) for v in range(KS)]
    terms += [(1, v, ("K8", v)) for v in range(KS) if ("xr", v) not in dropped]
    terms += [(0, v, ("Kr8", v)) for v in range(KS) if ("kr", v) not in dropped]

    def ok(a, b):
        return all(_pair_ok(a, b, pitch) for pitch in (J * WP, WP))

    pairs = []
    for parity in (0, 1):
        rem = sorted(
            [t for t in terms if t[1] % 2 == parity], key=lambda t: (t[0], t[1])
        )
        while rem:
            a = rem.pop(0)
            cand = next((i for i, b in enumerate(rem) if ok(a, b)), None)
            if cand is None:
                # zero-weight partner two columns away, kept in-bounds
                p, v, _ = a
                pairs.append((a, (p, v - 2 if v >= 2 else v + 2, None)))
            else:
                pairs.append((a, rem.pop(cand)))
    return pairs, K8, Kr8


def _pair_ok(a, b, pitch):
    off = lambda t: t[0] * pitch + t[1]
    s = abs(off(a) - off(b))
    return s % 2 == 0 and s not in (0, W)


def _build_weights(pairs, K8, Kr8):
    """fp8 DoubleRow lhsT stacks for the pair plan.

    Returns Wm [2(kind), npairs, 128, 2, MM] and We [npairs, 128, 2, 64].
    """
    def wcol(wsel):
        if wsel is None:
            return np.zeros(KS, np.float32)
        name, v = wsel
        return (K8 if name == "K8" else Kr8)[:, v]

    def stack(kind, M):
        out = np.zeros((len(pairs), 128, 2, M), np.float32)
        for i, pr in enumerate(pairs):
            # slot order must match the rhs AP: ascending (plane, v) offset
            pitch = J * WP if kind != "edge" else WP
            a, b = sorted(pr, key=lambda t: t[0] * pitch + t[1])
            out[i, :, 0, :] = _bands(wcol(a[2]), kind, M)
            out[i, :, 1, :] = _bands(wcol(b[2]), kind, M)
        return out.astype(F8NP)

    Wm = np.stack([stack("top", MM), stack("bot", MM)])
    We = stack("edge", 8 * EG)
    return Wm, We


def _build_nc(pairs=None, n_img=NIMG, xbufs=4, obufs=4, psbufs=5):
    if pairs is None:
        pairs = LAST_PAIRS
    assert pairs is not None, "call kernel() or pass pairs"
    npr = len(pairs)
    n_blk = n_img // J
    n_grp = n_img // EG
    nc = bacc.Bacc("TRN2", target_bir_lowering=False)
    Xm = nc.dram_tensor("Xm", [n_blk, 2, 128, 2 * J * WP], FP8, kind="ExternalInput")
    Xe = nc.dram_tensor("Xe", [n_grp, 128, 2 * WP], FP8, kind="ExternalInput")
    Wm = nc.dram_tensor("Wm", [128, 2 * npr, 2, MM], FP8, kind="ExternalInput")
    We = nc.dram_tensor("We", [128, npr, 2, 8 * EG], FP8, kind="ExternalInput")
    Om = nc.dram_tensor(
        "Om", [n_blk, 2, MT, J * W], mybir.dt.float32, kind="ExternalOutput"
    )
    Oe = nc.dram_tensor(
        "Oe", [n_grp, 8 * EG, W], mybir.dt.float32, kind="ExternalOutput"
    )

    def pair_rhs(xtile, pitch, img_off, pr, nimg=1):
        a, b = sorted(pr, key=lambda t: t[0] * pitch + t[1])
        o1 = img_off + a[0] * pitch + a[1]
        stride = (b[0] - a[0]) * pitch + (b[1] - a[1])
        assert stride > 0 and stride % 2 == 0 and stride != W, (a, b, stride)
        base = xtile[:]
        dims = [list(base.ap[0]), [stride, 2]]
        if nimg > 1:
            dims.append([WP, nimg])
        dims.append([1, W])
        return bass.AP(base.tensor, base.offset + o1, dims)

    with tile.TileContext(nc) as tc:
        with (
            tc.tile_pool(name="wpool", bufs=1) as wpool,
            tc.tile_pool(name="xpool", bufs=xbufs) as xpool,
            tc.tile_pool(name="epool", bufs=2) as epool,
            tc.tile_pool(name="opool", bufs=obufs) as opool,
            tc.tile_pool(name="oepool", bufs=2) as oepool,
            tc.tile_pool(name="psum", bufs=psbufs, space="PSUM") as pspool,
            tc.tile_pool(name="psum_e", bufs=2, space="PSUM") as pepool,
        ):
            wt = wpool.tile([128, 2 * npr, 2, MM], FP8)
            we = wpool.tile([128, npr, 2, 8 * EG], FP8)
            nc.gpsimd.dma_start(out=wt[:], in_=Wm[:])
            nc.gpsimd.dma_start(out=we[:], in_=We[:])

            for blk in range(n_blk):
                for half in range(2):
                    xt = xpool.tile([128, 2, J, WP], FP8)
                    nc.sync.dma_start(out=xt[:], in_=Xm[blk, half])
                    ot = opool.tile([MT, J * W], mybir.dt.float32)
                    wbase = half * npr
                    for j in range(J):
                        ps = pspool.tile([MM, W], mybir.dt.float32)
                        for i, pr in enumerate(pairs):
                            nc.tensor.matmul(
                                ps[:],
                                wt[:, wbase + i],
                                pair_rhs(xt, J * WP, j * WP, pr),
                                start=(i == 0),
                                stop=(i == npr - 1),
                                perf_mode=DR,
                            )
                        nc.vector.tensor_copy(ot[:, j * W : (j + 1) * W], ps[0:MT, :])
                    nc.scalar.dma_start(out=Om[blk, half], in_=ot[:])

                if blk % (EG // J) == 0:
                    g = blk // (EG // J)
                    et = epool.tile([128, 2, WP], FP8)
                    nc.gpsimd.dma_start(out=et[:], in_=Xe[g])
                    pse = pepool.tile([8 * EG, W], mybir.dt.float32)
                    for i, pr in enumerate(pairs):
                        nc.tensor.matmul(
                            pse[:],
                            we[:, i],
                            pair_rhs(et, WP, 0, pr),
                            start=(i == 0),
                            stop=(i == npr - 1),
                            perf_mode=DR,
                        )
                    oe = oepool.tile([8 * EG, W], mybir.dt.float32)
                    nc.vector.tensor_copy(oe[:], pse[:])
                    nc.gpsimd.dma_start(out=Oe[g], in_=oe[:])
    nc.compile()
    return nc


def _prep_inputs(X):
    """Host prep: fp8 split X = X8 + Xr8, pad width, pack per-tile layout.

    Returns Xm [cores, NBLK, 2, 128, 2*J*WP] and Xe [cores, NGRP, 128, 2*WP].
    """
    Xf = X.reshape(B * C, H, W)
    X8 = Xf.astype(F8NP)
    Xr8 = (Xf - X8.astype(np.float32)).astype(F8NP)

    p8 = np.zeros((2, B * C, H, WP), F8NP)
    p8[0, :, :, PAD : PAD + W] = X8
    p8[1, :, :, PAD : PAD + W] = Xr8
    # [2pl, cores, blk, J, half, 128, WP] -> [cores, blk, half, 128, 2pl, J, WP]
    Xm = (
        p8.reshape(2, N_CORES, NBLK, J, 2, 128, WP)
        .transpose(1, 2, 4, 5, 0, 3, 6)
        .reshape(N_CORES, NBLK, 2, 128, 2 * J * WP)
    )
    Xm = np.ascontiguousarray(Xm)
    # edge strips: rows 120..135 of each image, 8 images stacked per group
    Xe = np.ascontiguousarray(
        p8[:, :, 120:136, :]
        .reshape(2, N_CORES, NGRP, 128, WP)
        .transpose(1, 2, 3, 0, 4)
        .reshape(N_CORES, NGRP, 128, 2 * WP)
    )
    return Xm, Xe


def _assemble_output(res):
    """Reassemble [B, C, H, W] fp32 from per-core Om/Oe."""
    out = np.empty((N_CORES, NIMG, H, W), np.float32)
    for k in range(N_CORES):
        om = res.results[k]["Om"].reshape(NBLK, 2, MT, J, W)
        oe = res.results[k]["Oe"].reshape(NGRP * EG, 8, W)
        o = out[k].reshape(NBLK, J, H, W)
        o[:, :, 0:MT, :] = om[:, 0].transpose(0, 2, 1, 3)
        o[:, :, 132 : 132 + MT, :] = om[:, 1].transpose(0, 2, 1, 3)
        out[k][:, 124:132, :] = oe
    return out.reshape(B, C, H, W)


def kernel(X, K):
    global LAST_RESULT, LAST_PAIRS
    X = np.asarray(X)
    K = np.asarray(K)
    assert X.shape == (B, C, H, W) and K.shape == (1, 1, KS, KS)

    pairs, K8, Kr8 = plan_pairs(K[0, 0].astype(np.float32))
    # drop any pairing an hw constraint would reject (defensive; the
    # parity classes already guarantee even strides)
    for pitch in (J * WP, WP):
        assert all(_pair_ok(a, b, pitch) for a, b in pairs), pairs
    LAST_PAIRS = pairs

    Xm, Xe = _prep_inputs(X)
    Wm, We = _build_weights(pairs, K8, Kr8)
    Wmc = np.ascontiguousarray(
        Wm.reshape(2 * len(pairs), 128, 2, MM).transpose(1, 0, 2, 3)
    )
    Wec = np.ascontiguousarray(We.transpose(1, 0, 2, 3))

    nc = _build_nc(pairs)
    in_maps = [
        {"Xm": Xm[k], "Xe": Xe[k], "Wm": Wmc, "We": Wec}
        for k in range(N_CORES)
    ]
    res = run_bass_kernel_spmd(nc, in_maps, core_ids=list(range(N_CORES)))
    LAST_RESULT = res
    return _assemble_output(res)
